# revision 5
# baseline (speedup 1.0000x reference)
"""DialogueRNNCell fused Bass/Tile kernel for 8 Trainium2 NeuronCores.

Pure data parallel over the batch dim (2048 -> 256 rows/core). Per core, a
single fused pass over its g_hist shard computes the attention:
  scale_t = G_t . w       -- DVE affine_mul_reduce, or DVE mul + ACT accum
                             (split between engines to balance load)
  e_t = exp(scale_t)      -- ACT (exp table set)
  context += diag(e_t)^T @ G_t  -- tensor engine PSUM accumulation
The g GRU is interleaved into the attention loop (its sigmoids are computed
as 0.5+0.5*tanh(x/2) so the ACT engine stays on the exp/tanh table set).
The p/e GRU cells run after attention in transposed layout (features on
partitions) so the tensor engine accumulates gi+gh directly in PSUM and ACT
fuses bias+sigmoid/tanh. GRU combines are algebraically fused:
  h' = h + z'*(n - h),  z' = 1 - z;  party blend: res = q + m*z'*(n - q).
"""

import numpy as np
import ml_dtypes

import concourse.bass as bass
import concourse.mybir as mybir
import concourse.tile as tile
from concourse import bacc
from concourse.bass_utils import run_bass_kernel_spmd

B, T, NP, D = 2048, 128, 2, 512
N_CORES = 8
BL = B // N_CORES          # rows per core (256)
NBLK = BL // 128           # 128-row blocks per core (2)
NDC = D // 128             # feature chunks of 128 (4)
G3 = 3 * D                 # stacked gate rows (r, z, n) = 1536
NGC = G3 // 128            # gate chunks (12)

# scale-path split: tile idx % AMR_MOD < AMR_K -> affine_mul_reduce on DVE,
# else DVE mul + ACT copy-accum. Tuned so DVE and ACT finish together.
AMR_K, AMR_MOD = 15, 32

BF = mybir.dt.bfloat16
F32 = mybir.dt.float32
nbf = ml_dtypes.bfloat16

AX = mybir.AxisListType.X
ALU = mybir.AluOpType
ACTF = mybir.ActivationFunctionType

_compiled = {}


def _emit_gru(nc, sb, rz_psum, n_psum, wih, whh, x_chunks, h_chunks, bpack, bl):
    """One GRU cell in transposed layout (p/e GRUs, post-attention phase).

    bpack cols: 0:12 b_ih+b_hh, 12:24 -(b_ih+b_hh), 24:36 b_ih, 36:48 b_hh.
    Returns (zp_chunks, n_chunks).
    """

    def mm_chunk(c, klist, xlist, ps, first, last):
        n = len(klist)
        for i, (w, x) in enumerate(zip(klist, xlist)):
            nc.tensor.matmul(
                ps[:], w[:, c * 128:(c + 1) * 128], x[:],
                start=(first and i == 0), stop=(last and i == n - 1))

    r_sb, zp_sb, n_sb = [], [], []
    for j in range(4):                     # r gates: chunks 0..3
        c = j
        ps = rz_psum.tile([128, bl], F32, tag='gru_ps', name='gru_ps')
        mm_chunk(c, wih, x_chunks, ps, True, False)
        mm_chunk(c, whh, h_chunks, ps, False, True)
        r = sb.tile([128, bl], F32, tag=f'gru_r{j}', name=f'gru_r{j}')
        nc.scalar.activation(r[:], ps[:], ACTF.Sigmoid, bias=bpack[:, c:c + 1])
        r_sb.append(r)
    for j in range(4):                     # z gates: chunks 4..7 -> z' = 1-z
        c = 4 + j
        ps = rz_psum.tile([128, bl], F32, tag='gru_ps', name='gru_ps')
        mm_chunk(c, wih, x_chunks, ps, True, False)
        mm_chunk(c, whh, h_chunks, ps, False, True)
        zp = sb.tile([128, bl], F32, tag=f'gru_zp{j}', name=f'gru_zp{j}')
        nc.scalar.activation(zp[:], ps[:], ACTF.Sigmoid,
                             bias=bpack[:, 12 + c:12 + c + 1], scale=-1.0)
        zp_sb.append(zp)
    for j in range(4):                     # n gates: chunks 8..11
        c = 8 + j
        ps_i = n_psum.tile([128, bl], F32, tag='gru_psn_i', name='gru_psn_i')
        mm_chunk(c, wih, x_chunks, ps_i, True, True)
        ps_h = n_psum.tile([128, bl], F32, tag='gru_psn_h', name='gru_psn_h')
        mm_chunk(c, whh, h_chunks, ps_h, True, True)
        tmp = sb.tile([128, bl], F32, tag='gru_tmp', name='gru_tmp')
        nc.vector.scalar_tensor_tensor(        # (gh_n + b_hh) * r
            tmp[:], ps_h[:], bpack[:, 36 + c:36 + c + 1], r_sb[j][:],
            ALU.add, ALU.mult)
        tmp2 = sb.tile([128, bl], F32, tag='gru_tmp2', name='gru_tmp2')
        nc.vector.tensor_add(tmp2[:], tmp[:], ps_i[:])
        ng = sb.tile([128, bl], F32, tag=f'gru_n{j}', name=f'gru_n{j}')
        nc.scalar.activation(ng[:], tmp2[:], ACTF.Tanh,
                             bias=bpack[:, 24 + c:24 + c + 1])
        n_sb.append(ng)
    return zp_sb, n_sb


def _build(loop_n=1):
    nc = bacc.Bacc('TRN2', target_bir_lowering=False, debug=False,
                   num_devices=N_CORES)
    # ---- inputs (per-core shapes) ----
    g = nc.dram_tensor('g', [T, BL, D], BF, kind='ExternalInput')
    wb = nc.dram_tensor('wb', [128, D], BF, kind='ExternalInput')
    ident = nc.dram_tensor('ident', [128, 128], BF, kind='ExternalInput')
    utT = nc.dram_tensor('utT', [D, BL], BF, kind='ExternalInput')
    q0T32 = nc.dram_tensor('q0T32', [D, BL], F32, kind='ExternalInput')
    q1T32 = nc.dram_tensor('q1T32', [D, BL], F32, kind='ExternalInput')
    q0Tbf = nc.dram_tensor('q0Tbf', [D, BL], BF, kind='ExternalInput')
    q1Tbf = nc.dram_tensor('q1Tbf', [D, BL], BF, kind='ExternalInput')
    eT32 = nc.dram_tensor('eT32', [D, BL], F32, kind='ExternalInput')
    eTbf = nc.dram_tensor('eTbf', [D, BL], BF, kind='ExternalInput')
    hT32 = nc.dram_tensor('hT32', [D, BL], F32, kind='ExternalInput')
    hTbf = nc.dram_tensor('hTbf', [D, BL], BF, kind='ExternalInput')
    qm0 = nc.dram_tensor('qm0', [1, BL], F32, kind='ExternalInput')
    qm1 = nc.dram_tensor('qm1', [1, BL], F32, kind='ExternalInput')
    wdecl = {}
    for nm, kk in [('gwih', 8), ('gwhh', 4), ('pwih', 8), ('pwhh', 4),
                   ('ewih', 4), ('ewhh', 4)]:
        wdecl[nm] = nc.dram_tensor(nm, [kk * 128, G3], BF, kind='ExternalInput')
    # bias packs [128, 72]: 0:12 s, 12:24 -s, 24:36 b_ih, 36:48 b_hh,
    #                       48:60 s/2, 60:72 -s/2   (s = b_ih + b_hh)
    bdecl = {nm: nc.dram_tensor(nm, [128, 72], F32, kind='ExternalInput')
             for nm in ('bg', 'bp', 'be')}
    # ---- outputs ----
    att_o = nc.dram_tensor('att_o', [BL, T], F32, kind='ExternalOutput')
    rgT_o = nc.dram_tensor('rgT_o', [D, BL], F32, kind='ExternalOutput')
    rqT_o = nc.dram_tensor('rqT_o', [NP, D, BL], F32, kind='ExternalOutput')
    reT_o = nc.dram_tensor('reT_o', [D, BL], F32, kind='ExternalOutput')

    from contextlib import ExitStack
    with tile.TileContext(nc) as tc, ExitStack() as top:
        const = top.enter_context(tc.tile_pool(name='const', bufs=1))
        sb = top.enter_context(tc.tile_pool(name='sb', bufs=1))
        work = top.enter_context(tc.tile_pool(name='work', bufs=3))

        # ---- persistent constants / weights ----
        wb_sb = const.tile([128, D], BF, tag='wb', name='wb')
        nc.sync.dma_start(wb_sb[:], wb[:])
        I_sb = const.tile([128, 128], BF, tag='ident', name='ident')
        nc.sync.dma_start(I_sb[:], ident[:])
        wtiles = {}
        for nm, kk in [('gwih', 8), ('gwhh', 4), ('pwih', 8), ('pwhh', 4)]:
            tl = []
            for k in range(kk):
                w = const.tile([128, G3], BF, tag=f'{nm}{k}', name=f'{nm}{k}')
                nc.sync.dma_start(w[:], wdecl[nm][k * 128:(k + 1) * 128, :])
                tl.append(w)
            wtiles[nm] = tl
        btiles = {}
        for nm in ('bg', 'bp', 'be'):
            bt = const.tile([128, 72], F32, tag=nm)
            nc.sync.dma_start(bt[:], bdecl[nm][:])
            btiles[nm] = bt

        def load_T(dram, dt, tagp):
            tl = []
            for j in range(NDC):
                x = const.tile([128, BL], dt, tag=f'{tagp}{j}', name=f'{tagp}{j}')
                nc.sync.dma_start(x[:], dram[j * 128:(j + 1) * 128, :])
                tl.append(x)
            return tl

        uT = load_T(utT, BF, 'uT')
        q0_32 = load_T(q0T32, F32, 'q032')
        q1_32 = load_T(q1T32, F32, 'q132')
        q0_bf = load_T(q0Tbf, BF, 'q0bf')
        q1_bf = load_T(q1Tbf, BF, 'q1bf')
        e_32 = load_T(eT32, F32, 'e32')
        e_bf = load_T(eTbf, BF, 'ebf')
        h_32 = load_T(hT32, F32, 'h32')
        h_bf = load_T(hTbf, BF, 'hbf')

        qm0_sb = const.tile([1, BL], F32, tag='qm0', name='qm0')
        nc.sync.dma_start(qm0_sb[:], qm0[:])
        qm1_sb = const.tile([1, BL], F32, tag='qm1', name='qm1')
        nc.sync.dma_start(qm1_sb[:], qm1[:])
        is1_row = const.tile([1, BL], F32, tag='is1row', name='is1row')
        nc.vector.tensor_tensor(out=is1_row[:], in0=qm1_sb[:], in1=qm0_sb[:],
                                op=ALU.is_gt)
        is1_b = const.tile([128, BL], F32, tag='is1b', name='is1b')
        nc.gpsimd.partition_broadcast(is1_b[:], is1_row[:])
        m0_b = const.tile([128, BL], F32, tag='m0b', name='m0b')
        nc.gpsimd.partition_broadcast(m0_b[:], qm0_sb[:])
        m1_b = const.tile([128, BL], F32, tag='m1b', name='m1b')
        nc.gpsimd.partition_broadcast(m1_b[:], qm1_sb[:])

        for _rep in range(loop_n):
            # q0_sel^T = q0 + is1*(q1 - q0); cast bf16 (g GRU needs it early)
            qsel_bf = []
            for j in range(NDC):
                u = work.tile([128, BL], F32, tag='wu', name='wu')
                nc.vector.tensor_sub(u[:], q1_32[j][:], q0_32[j][:])
                v = work.tile([128, BL], F32, tag='wv', name='wv')
                nc.vector.tensor_mul(v[:], is1_b[:], u[:])
                w32 = work.tile([128, BL], F32, tag='ww', name='ww')
                nc.vector.tensor_add(w32[:], q0_32[j][:], v[:])
                wbf = sb.tile([128, BL], BF, tag=f'qselbf{j}', name=f'qselbf{j}')
                nc.vector.tensor_copy(wbf[:], w32[:])
                qsel_bf.append(wbf)

            # ============ attention + interleaved g GRU ============
            e_sb = [sb.tile([128, T], F32, tag=f'esb{b}', name=f'esb{b}')
                    for b in range(NBLK)]
            sc_sb = [sb.tile([128, T], F32, tag=f'scsb{b}', name=f'scsb{b}')
                     for b in range(NBLK)]
            ctxT_bf = [sb.tile([128, BL], BF, tag=f'ctxT{j}', name=f'ctxT{j}')
                       for j in range(NDC)]
            g_zp, g_ng, g_r = [None] * 4, [None] * 4, [None] * 4

            gx = uT + qsel_bf          # g GRU x rhs chunks (K=1024)
            bg = btiles['bg']

            def g_gru_chunk(ci, gps, nps):
                """Emit g-GRU gate-chunk ci inside the attention loop.
                Sigmoids via tanh: sigma(x) = 0.5 + 0.5*tanh(x/2)."""
                def mms(c, ps, klist, xlist, first, last):
                    n = len(klist)
                    for i, (w, x) in enumerate(zip(klist, xlist)):
                        nc.tensor.matmul(
                            ps[:], w[:, c * 128:(c + 1) * 128], x[:],
                            start=(first and i == 0), stop=(last and i == n - 1))
                if ci < 8:            # r (0..3) / z (4..7) gates
                    c = ci
                    ps = gps.tile([128, BL], F32, tag='ga_ps', name='ga_ps')
                    mms(c, ps, wtiles['gwih'], gx, True, False)
                    mms(c, ps, wtiles['gwhh'], h_bf, False, True)
                    th = work.tile([128, BL], F32, tag='ga_th', name='ga_th')
                    if ci < 4:
                        nc.scalar.activation(th[:], ps[:], ACTF.Tanh,
                                             bias=bg[:, 48 + c:48 + c + 1],
                                             scale=0.5)
                        r = sb.tile([128, BL], F32, tag=f'ga_r{ci}',
                                    name=f'ga_r{ci}')
                        nc.vector.tensor_scalar(out=r[:], in0=th[:],
                                                scalar1=0.5, scalar2=0.5,
                                                op0=ALU.mult, op1=ALU.add)
                        g_r[ci] = r
                    else:
                        nc.scalar.activation(th[:], ps[:], ACTF.Tanh,
                                             bias=bg[:, 60 + c:60 + c + 1],
                                             scale=-0.5)
                        zp = sb.tile([128, BL], F32, tag=f'ga_zp{ci - 4}',
                                     name=f'ga_zp{ci - 4}')
                        nc.vector.tensor_scalar(out=zp[:], in0=th[:],
                                                scalar1=0.5, scalar2=0.5,
                                                op0=ALU.mult, op1=ALU.add)
                        g_zp[ci - 4] = zp
                else:                 # n gate
                    c = ci
                    j = ci - 8
                    ps_i = nps.tile([128, BL], F32, tag='ga_psn_i',
                                    name='ga_psn_i')
                    mms(c, ps_i, wtiles['gwih'], gx, True, True)
                    ps_h = nps.tile([128, BL], F32, tag='ga_psn_h',
                                    name='ga_psn_h')
                    mms(c, ps_h, wtiles['gwhh'], h_bf, True, True)
                    tmp = work.tile([128, BL], F32, tag='ga_tmp', name='ga_tmp')
                    nc.vector.scalar_tensor_tensor(
                        tmp[:], ps_h[:], bg[:, 36 + c:36 + c + 1], g_r[j][:],
                        ALU.add, ALU.mult)
                    tmp2 = work.tile([128, BL], F32, tag='ga_tmp2',
                                     name='ga_tmp2')
                    nc.vector.tensor_add(tmp2[:], tmp[:], ps_i[:])
                    ng = sb.tile([128, BL], F32, tag=f'ga_n{j}', name=f'ga_n{j}')
                    nc.scalar.activation(ng[:], tmp2[:], ACTF.Tanh,
                                         bias=bg[:, 24 + c:24 + c + 1])
                    g_ng[j] = ng

            with tc.tile_pool(name='cpsum', bufs=1, space='PSUM') as cpsum, \
                 tc.tile_pool(name='gapsum', bufs=2, space='PSUM') as gapsum, \
                 tc.tile_pool(name='ganpsum', bufs=1, space='PSUM') as ganpsum, \
                 tc.tile_pool(name='gp', bufs=5) as gp, \
                 tc.tile_pool(name='yp', bufs=3) as yp, \
                 tc.tile_pool(name='ys', bufs=2) as ys, \
                 tc.tile_pool(name='dgp', bufs=3) as dgp:
                psum_c = [cpsum.tile([128, D], F32, tag=f'c{b}', name=f'c{b}')
                          for b in range(NBLK)]
                gbig = None
                gsl = [[None] * 4 for _ in range(NBLK)]
                gchunk_sched = {8 + ci * 9: ci for ci in range(NGC)}
                for t in range(T):
                    ti = t % 4
                    if t % 2 == 0:
                        gbig = gp.tile([128, 4 * D], BF, tag='g', name='g')
                        nc.sync.dma_start(
                            gbig[:].rearrange('p (u v d) -> p u v d', u=2, v=2),
                            g[t:t + 2, :, :].rearrange('u (v p) d -> p u v d',
                                                       v=2))
                    for b in range(NBLK):
                        gt = gbig[:, ((t % 2) * 2 + b) * D:
                                  ((t % 2) * 2 + b + 1) * D]
                        gsl[b][ti] = gt
                        idx = t * NBLK + b
                        y = yp.tile([128, D], BF, tag=f'y{b}', name=f'y{b}')
                        if idx % AMR_MOD < AMR_K:
                            nc.vector.affine_mul_reduce(
                                out=y[:], accum_out=sc_sb[b][:, t:t + 1],
                                in0=gt, in1=wb_sb[:], scale=1.0, bias=0.0)
                        else:
                            nc.vector.tensor_mul(y[:], gt, wb_sb[:])
                            sink = ys.tile([128, D], BF, tag=f'ys{b}',
                                           name=f'ys{b}')
                            nc.scalar.activation(
                                sink[:], y[:], ACTF.Copy,
                                accum_out=sc_sb[b][:, t:t + 1])
                    if ti == 3:
                        for b in range(NBLK):
                            nc.scalar.activation(
                                e_sb[b][:, t - 3:t + 1],
                                sc_sb[b][:, t - 3:t + 1], ACTF.Exp)
                            for k in range(4):
                                tt = t - 3 + k
                                dg = dgp.tile([128, 128], BF, tag=f'd{b}',
                                              name=f'd{b}')
                                nc.vector.tensor_scalar_mul(
                                    dg[:], I_sb[:], e_sb[b][:, tt:tt + 1])
                                nc.tensor.matmul(
                                    psum_c[b][:], dg[:], gsl[b][k],
                                    start=(tt == 0), stop=(tt == T - 1))
                    if t in gchunk_sched:
                        g_gru_chunk(gchunk_sched[t], gapsum, ganpsum)

                # softmax normalize; att out; context -> transposed bf16
                with tc.tile_pool(name='tp', bufs=2, space='PSUM') as tpsum:
                    for b in range(NBLK):
                        s = sb.tile([128, 1], F32, tag=f's{b}', name=f's{b}')
                        nc.vector.reduce_sum(s[:], e_sb[b][:], axis=AX)
                        rcp = sb.tile([128, 1], F32, tag=f'rcp{b}',
                                      name=f'rcp{b}')
                        nc.vector.reciprocal(rcp[:], s[:])
                        att_sb = sb.tile([128, T], F32, tag=f'att{b}',
                                         name=f'att{b}')
                        nc.vector.tensor_scalar_mul(att_sb[:], e_sb[b][:],
                                                    rcp[:])
                        nc.sync.dma_start(att_o[b * 128:(b + 1) * 128, :],
                                          att_sb[:])
                        ctx_f = sb.tile([128, D], F32, tag=f'ctxf{b}',
                                        name=f'ctxf{b}')
                        nc.vector.tensor_scalar_mul(ctx_f[:], psum_c[b][:],
                                                    rcp[:])
                        ctx_b = sb.tile([128, D], BF, tag=f'ctxb{b}',
                                        name=f'ctxb{b}')
                        nc.vector.tensor_copy(ctx_b[:], ctx_f[:])
                        for j in range(NDC):
                            pt = tpsum.tile([128, 128], BF, tag='tr', name='tr')
                            nc.tensor.transpose(
                                pt[:], ctx_b[:, j * 128:(j + 1) * 128], I_sb[:])
                            nc.vector.tensor_copy(
                                ctxT_bf[j][:, b * 128:(b + 1) * 128], pt[:])

            # g GRU combine: res_g = h + z'*(n - h)
            for j in range(NDC):
                u = work.tile([128, BL], F32, tag='wu', name='wu')
                nc.vector.tensor_sub(u[:], g_ng[j][:], h_32[j][:])
                v = work.tile([128, BL], F32, tag='wv', name='wv')
                nc.vector.tensor_mul(v[:], g_zp[j][:], u[:])
                r = work.tile([128, BL], F32, tag='ww', name='ww')
                nc.vector.tensor_add(r[:], h_32[j][:], v[:])
                nc.sync.dma_start(rgT_o[j * 128:(j + 1) * 128, :], r[:])

            # ==================== p / e GRU phase ====================
            with tc.tile_pool(name='gpsum', bufs=3, space='PSUM') as gpsum, \
                 tc.tile_pool(name='npsum', bufs=2, space='PSUM') as npsum, \
                 tc.tile_pool(name='ewp', bufs=1) as ewp:
                for nm in ('ewih', 'ewhh'):
                    tl = []
                    for k in range(4):
                        w = ewp.tile([128, G3], BF, tag=f'{nm}{k}',
                                     name=f'{nm}{k}')
                        nc.sync.dma_start(w[:],
                                          wdecl[nm][k * 128:(k + 1) * 128, :])
                        tl.append(w)
                    wtiles[nm] = tl
                res_p = [[None] * NDC for _ in range(NP)]
                for p in range(NP):
                    hbf = q0_bf if p == 0 else q1_bf
                    h32 = q0_32 if p == 0 else q1_32
                    mb = m0_b if p == 0 else m1_b
                    zp, ng = _emit_gru(nc, sb, gpsum, npsum, wtiles['pwih'],
                                       wtiles['pwhh'], uT + ctxT_bf, hbf,
                                       btiles['bp'], BL)
                    for j in range(NDC):
                        u = work.tile([128, BL], F32, tag='wu', name='wu')
                        nc.vector.tensor_sub(u[:], ng[j][:], h32[j][:])
                        v = work.tile([128, BL], F32, tag='wv', name='wv')
                        nc.vector.tensor_mul(v[:], zp[j][:], u[:])
                        w2 = work.tile([128, BL], F32, tag='ww', name='ww')
                        nc.vector.tensor_mul(w2[:], mb[:], v[:])
                        r = sb.tile([128, BL], F32, tag=f'resp{p}{j}',
                                    name=f'resp{p}{j}')
                        nc.vector.tensor_add(r[:], h32[j][:], w2[:])
                        res_p[p][j] = r
                        nc.sync.dma_start(rqT_o[p, j * 128:(j + 1) * 128, :],
                                          r[:])

                # sel^T = res0 + is1*(res1 - res0); cast bf16
                sel_bf = []
                for j in range(NDC):
                    u = work.tile([128, BL], F32, tag='wu', name='wu')
                    nc.vector.tensor_sub(u[:], res_p[1][j][:], res_p[0][j][:])
                    v = work.tile([128, BL], F32, tag='wv', name='wv')
                    nc.vector.tensor_mul(v[:], is1_b[:], u[:])
                    w32 = work.tile([128, BL], F32, tag='ww', name='ww')
                    nc.vector.tensor_add(w32[:], res_p[0][j][:], v[:])
                    wbf = sb.tile([128, BL], BF, tag=f'selbf{j}',
                                  name=f'selbf{j}')
                    nc.vector.tensor_copy(wbf[:], w32[:])
                    sel_bf.append(wbf)

                # ---- e GRU: x=sel, h=e; res = e + z'(n-e)
                zp, ng = _emit_gru(nc, sb, gpsum, npsum, wtiles['ewih'],
                                   wtiles['ewhh'], sel_bf, e_bf,
                                   btiles['be'], BL)
                for j in range(NDC):
                    u = work.tile([128, BL], F32, tag='wu', name='wu')
                    nc.vector.tensor_sub(u[:], ng[j][:], e_32[j][:])
                    v = work.tile([128, BL], F32, tag='wv', name='wv')
                    nc.vector.tensor_mul(v[:], zp[j][:], u[:])
                    r = work.tile([128, BL], F32, tag='ww', name='ww')
                    nc.vector.tensor_add(r[:], e_32[j][:], v[:])
                    nc.sync.dma_start(reT_o[j * 128:(j + 1) * 128, :], r[:])

    nc.compile()
    return nc


def _host_prep(utter, q_mask, g_hist, q, e, w_att,
               g_w_ih, g_w_hh, g_b_ih, g_b_hh,
               p_w_ih, p_w_hh, p_b_ih, p_b_hh,
               e_w_ih, e_w_hh, e_b_ih, e_b_hh):
    """Build per-core input maps (layout marshalling only, no model math)."""
    g_bf = np.ascontiguousarray(g_hist).astype(nbf)
    wbc = np.broadcast_to(np.asarray(w_att).astype(nbf), (128, D)).copy()
    ident = np.eye(128, dtype=nbf)

    def packT(x):
        xt = np.ascontiguousarray(np.asarray(x).T)
        return xt.astype(np.float32), xt.astype(nbf)

    q0_32, q0_bf = packT(q[:, 0, :])
    q1_32, q1_bf = packT(q[:, 1, :])
    e_32, e_bf = packT(e)
    h_32, h_bf = packT(np.asarray(g_hist[-1]))
    uT_bf = np.ascontiguousarray(np.asarray(utter).T).astype(nbf)
    qmT = np.ascontiguousarray(np.asarray(q_mask).T).astype(np.float32)

    def wT(w):
        return np.ascontiguousarray(np.asarray(w).T).astype(nbf)

    def bias_pack(b_ih, b_hh):
        s = (np.asarray(b_ih) + np.asarray(b_hh)).reshape(NGC, 128).T
        bi = np.asarray(b_ih).reshape(NGC, 128).T
        bh = np.asarray(b_hh).reshape(NGC, 128).T
        out = np.zeros((128, 72), np.float32)
        out[:, 0:12] = s
        out[:, 12:24] = -s
        out[:, 24:36] = bi
        out[:, 36:48] = bh
        out[:, 48:60] = 0.5 * s
        out[:, 60:72] = -0.5 * s
        return out

    shared = {
        'wb': wbc, 'ident': ident,
        'gwih': wT(g_w_ih), 'gwhh': wT(g_w_hh),
        'pwih': wT(p_w_ih), 'pwhh': wT(p_w_hh),
        'ewih': wT(e_w_ih), 'ewhh': wT(e_w_hh),
        'bg': bias_pack(g_b_ih, g_b_hh),
        'bp': bias_pack(p_b_ih, p_b_hh),
        'be': bias_pack(e_b_ih, e_b_hh),
    }
    in_maps = []
    for c in range(N_CORES):
        sl = slice(c * BL, (c + 1) * BL)
        m = dict(shared)
        m.update({
            'g': np.ascontiguousarray(g_bf[:, sl, :]),
            'utT': np.ascontiguousarray(uT_bf[:, sl]),
            'q0T32': np.ascontiguousarray(q0_32[:, sl]),
            'q1T32': np.ascontiguousarray(q1_32[:, sl]),
            'q0Tbf': np.ascontiguousarray(q0_bf[:, sl]),
            'q1Tbf': np.ascontiguousarray(q1_bf[:, sl]),
            'eT32': np.ascontiguousarray(e_32[:, sl]),
            'eTbf': np.ascontiguousarray(e_bf[:, sl]),
            'hT32': np.ascontiguousarray(h_32[:, sl]),
            'hTbf': np.ascontiguousarray(h_bf[:, sl]),
            'qm0': np.ascontiguousarray(qmT[0:1, sl]),
            'qm1': np.ascontiguousarray(qmT[1:2, sl]),
        })
        in_maps.append(m)
    return in_maps


def kernel(**inputs):
    inputs = {k: np.asarray(v) for k, v in inputs.items()}
    in_maps = _host_prep(**inputs)
    if 'nc' not in _compiled:
        _compiled['nc'] = _build()
    nc = _compiled['nc']
    res = run_bass_kernel_spmd(nc, in_maps, list(range(N_CORES))).results

    res_g = np.concatenate([r['rgT_o'].T for r in res], axis=0)
    res_q = np.concatenate([np.transpose(r['rqT_o'], (2, 0, 1)) for r in res],
                           axis=0)
    res_e = np.concatenate([r['reT_o'].T for r in res], axis=0)
    att = np.concatenate([r['att_o'][:, None, :] for r in res], axis=0)
    return (np.ascontiguousarray(res_g, dtype=np.float32),
            np.ascontiguousarray(res_q, dtype=np.float32),
            np.ascontiguousarray(res_e, dtype=np.float32),
            np.ascontiguousarray(att, dtype=np.float32))


# revision 11
# speedup vs baseline: 1.0340x; 1.0340x over previous
"""DialogueRNNCell fused Bass/Tile kernel for 8 Trainium2 NeuronCores.

Pure data parallel over the batch dim (2048 -> 256 rows/core). Per core, a
single fused pass over its g_hist shard computes the attention:
  scale_t = G_t . w       -- DVE affine_mul_reduce, or DVE mul + ACT accum
                             (split between engines to balance load)
  e_t = exp(scale_t)      -- ACT (exp table set)
  context += diag(e_t)^T @ G_t  -- tensor engine PSUM accumulation
The g GRU is interleaved into the attention loop (its sigmoids are computed
as 0.5+0.5*tanh(x/2) so the ACT engine stays on the exp/tanh table set).
The p/e GRU cells run after attention in transposed layout (features on
partitions) so the tensor engine accumulates gi+gh directly in PSUM and ACT
fuses bias+sigmoid/tanh. GRU combines are algebraically fused:
  h' = h + z'*(n - h),  z' = 1 - z;  party blend: res = q + m*z'*(n - q).
"""

import numpy as np
import ml_dtypes

import concourse.bass as bass
import concourse.mybir as mybir
import concourse.tile as tile
from concourse import bacc
from concourse.bass_utils import run_bass_kernel_spmd

B, T, NP, D = 2048, 128, 2, 512
N_CORES = 8
BL = B // N_CORES          # rows per core (256)
NBLK = BL // 128           # 128-row blocks per core (2)
NDC = D // 128             # feature chunks of 128 (4)
G3 = 3 * D                 # stacked gate rows (r, z, n) = 1536
NGC = G3 // 128            # gate chunks (12)

# scale-path split: tile idx % AMR_MOD < AMR_K -> affine_mul_reduce on DVE,
# else DVE mul + ACT copy-accum. Tuned so DVE and ACT finish together.
AMR_K, AMR_MOD = 6, 13

BF = mybir.dt.bfloat16
F32 = mybir.dt.float32
nbf = ml_dtypes.bfloat16

AX = mybir.AxisListType.X
ALU = mybir.AluOpType
ACTF = mybir.ActivationFunctionType

_compiled = {}


def _emit_gru(nc, sb, rz_psum, n_psum, wih, whh, x_chunks, h_chunks, bpack, bl):
    """One GRU cell in transposed layout (p/e GRUs, post-attention phase).

    bpack cols: 0:12 b_ih+b_hh, 12:24 -(b_ih+b_hh), 24:36 b_ih, 36:48 b_hh.
    Returns (zp_chunks, n_chunks).
    """

    def mm_chunk(c, klist, xlist, ps, first, last):
        n = len(klist)
        for i, (w, x) in enumerate(zip(klist, xlist)):
            xa = x if isinstance(x, bass.AP) else x[:]
            nc.tensor.matmul(
                ps[:], w[:, c * 128:(c + 1) * 128], xa,
                start=(first and i == 0), stop=(last and i == n - 1))

    r_sb, zp_sb, n_sb = [], [], []
    for j in range(4):                     # r gates: chunks 0..3
        c = j
        ps = rz_psum.tile([128, bl], F32, tag='gru_ps', name='gru_ps')
        mm_chunk(c, wih, x_chunks, ps, True, False)
        mm_chunk(c, whh, h_chunks, ps, False, True)
        r = sb.tile([128, bl], F32, tag=f'gru_r{j}', name=f'gru_r{j}')
        nc.scalar.activation(r[:], ps[:], ACTF.Sigmoid, bias=bpack[:, c:c + 1])
        r_sb.append(r)
    for j in range(4):                     # z gates: chunks 4..7 -> z' = 1-z
        c = 4 + j
        ps = rz_psum.tile([128, bl], F32, tag='gru_ps', name='gru_ps')
        mm_chunk(c, wih, x_chunks, ps, True, False)
        mm_chunk(c, whh, h_chunks, ps, False, True)
        zp = sb.tile([128, bl], F32, tag=f'gru_zp{j}', name=f'gru_zp{j}')
        nc.scalar.activation(zp[:], ps[:], ACTF.Sigmoid,
                             bias=bpack[:, 12 + c:12 + c + 1], scale=-1.0)
        zp_sb.append(zp)
    for j in range(4):                     # n gates: chunks 8..11
        c = 8 + j
        ps_i = n_psum.tile([128, bl], F32, tag='gru_psn_i', name='gru_psn_i')
        mm_chunk(c, wih, x_chunks, ps_i, True, True)
        ps_h = n_psum.tile([128, bl], F32, tag='gru_psn_h', name='gru_psn_h')
        mm_chunk(c, whh, h_chunks, ps_h, True, True)
        tmp = sb.tile([128, bl], F32, tag='gru_tmp', name='gru_tmp')
        nc.vector.scalar_tensor_tensor(        # (gh_n + b_hh) * r
            tmp[:], ps_h[:], bpack[:, 36 + c:36 + c + 1], r_sb[j][:],
            ALU.add, ALU.mult)
        tmp2 = sb.tile([128, bl], F32, tag='gru_tmp2', name='gru_tmp2')
        nc.vector.tensor_add(tmp2[:], tmp[:], ps_i[:])
        ng = sb.tile([128, bl], F32, tag=f'gru_n{j}', name=f'gru_n{j}')
        nc.scalar.activation(ng[:], tmp2[:], ACTF.Tanh,
                             bias=bpack[:, 24 + c:24 + c + 1])
        n_sb.append(ng)
    return zp_sb, n_sb


def _build(loop_n=1):
    nc = bacc.Bacc('TRN2', target_bir_lowering=False, debug=False,
                   num_devices=N_CORES)
    # ---- inputs (per-core shapes) ----
    g = nc.dram_tensor('g', [T, BL, D], BF, kind='ExternalInput')
    wb = nc.dram_tensor('wb', [128, D], BF, kind='ExternalInput')
    ident = nc.dram_tensor('ident', [128, 128], BF, kind='ExternalInput')
    utT2 = nc.dram_tensor('utT2', [D, 2 * BL], BF, kind='ExternalInput')
    qpTbf = nc.dram_tensor('qpTbf', [D, 2 * BL], BF, kind='ExternalInput')
    q0T32 = nc.dram_tensor('q0T32', [D, BL], F32, kind='ExternalInput')
    q1T32 = nc.dram_tensor('q1T32', [D, BL], F32, kind='ExternalInput')
    eT32 = nc.dram_tensor('eT32', [D, BL], F32, kind='ExternalInput')
    eTbf = nc.dram_tensor('eTbf', [D, BL], BF, kind='ExternalInput')
    hT32 = nc.dram_tensor('hT32', [D, BL], F32, kind='ExternalInput')
    hTbf = nc.dram_tensor('hTbf', [D, BL], BF, kind='ExternalInput')
    qm0 = nc.dram_tensor('qm0', [1, BL], F32, kind='ExternalInput')
    qm1 = nc.dram_tensor('qm1', [1, BL], F32, kind='ExternalInput')
    wdecl = {}
    for nm, kk in [('gwih', 8), ('gwhh', 4), ('pwih', 8), ('pwhh', 4),
                   ('ewih', 4), ('ewhh', 4)]:
        wdecl[nm] = nc.dram_tensor(nm, [kk * 128, G3], BF, kind='ExternalInput')
    # bias packs [128, 72]: 0:12 s, 12:24 -s, 24:36 b_ih, 36:48 b_hh,
    #                       48:60 s/2, 60:72 -s/2   (s = b_ih + b_hh)
    bdecl = {nm: nc.dram_tensor(nm, [128, 72], F32, kind='ExternalInput')
             for nm in ('bg', 'bp', 'be')}
    # ---- outputs ----
    att_o = nc.dram_tensor('att_o', [BL, T], F32, kind='ExternalOutput')
    rgT_o = nc.dram_tensor('rgT_o', [D, BL], F32, kind='ExternalOutput')
    rqT_o = nc.dram_tensor('rqT_o', [NP, D, BL], F32, kind='ExternalOutput')
    reT_o = nc.dram_tensor('reT_o', [D, BL], F32, kind='ExternalOutput')

    from contextlib import ExitStack
    with tile.TileContext(nc) as tc, ExitStack() as top:
        const = top.enter_context(tc.tile_pool(name='const', bufs=1))
        sb = top.enter_context(tc.tile_pool(name='sb', bufs=1))
        work = top.enter_context(tc.tile_pool(name='work', bufs=2))

        # ---- persistent constants / weights ----
        wb_sb = const.tile([128, D], BF, tag='wb', name='wb')
        nc.sync.dma_start(wb_sb[:], wb[:])
        I_sb = const.tile([128, 128], BF, tag='ident', name='ident')
        nc.sync.dma_start(I_sb[:], ident[:])
        wtiles = {}
        for nm, kk in [('gwih', 8), ('gwhh', 4)]:
            tl = []
            for k in range(kk):
                w = const.tile([128, G3], BF, tag=f'{nm}{k}', name=f'{nm}{k}')
                nc.sync.dma_start(w[:], wdecl[nm][k * 128:(k + 1) * 128, :])
                tl.append(w)
            wtiles[nm] = tl
        btiles = {}
        for nm in ('bg', 'bp', 'be'):
            bt = const.tile([128, 72], F32, tag=nm)
            nc.sync.dma_start(bt[:], bdecl[nm][:])
            btiles[nm] = bt

        def load_T(dram, dt, tagp):
            tl = []
            for j in range(NDC):
                x = const.tile([128, BL], dt, tag=f'{tagp}{j}', name=f'{tagp}{j}')
                nc.sync.dma_start(x[:], dram[j * 128:(j + 1) * 128, :])
                tl.append(x)
            return tl

        def load_T2(dram, dt, tagp):
            tl = []
            for j in range(NDC):
                x = const.tile([128, 2 * BL], dt, tag=f'{tagp}{j}',
                               name=f'{tagp}{j}')
                nc.sync.dma_start(x[:], dram[j * 128:(j + 1) * 128, :])
                tl.append(x)
            return tl

        ut2 = load_T2(utT2, BF, 'ut2')
        qp_bf = load_T2(qpTbf, BF, 'qpbf')
        uT = [x[:, 0:BL] for x in ut2]
        q0_32 = load_T(q0T32, F32, 'q032')
        q1_32 = load_T(q1T32, F32, 'q132')
        e_bf = load_T(eTbf, BF, 'ebf')
        h_32 = load_T(hT32, F32, 'h32')
        h_bf = load_T(hTbf, BF, 'hbf')

        qm0_sb = const.tile([1, BL], F32, tag='qm0', name='qm0')
        nc.sync.dma_start(qm0_sb[:], qm0[:])
        qm1_sb = const.tile([1, BL], F32, tag='qm1', name='qm1')
        nc.sync.dma_start(qm1_sb[:], qm1[:])
        is1_row = const.tile([1, BL], F32, tag='is1row', name='is1row')
        nc.vector.tensor_tensor(out=is1_row[:], in0=qm1_sb[:], in1=qm0_sb[:],
                                op=ALU.is_gt)
        is1_b = const.tile([128, BL], F32, tag='is1b', name='is1b')
        nc.gpsimd.partition_broadcast(is1_b[:], is1_row[:])
        m0_b = const.tile([128, BL], F32, tag='m0b', name='m0b')
        nc.gpsimd.partition_broadcast(m0_b[:], qm0_sb[:])
        m1_b = const.tile([128, BL], F32, tag='m1b', name='m1b')
        nc.gpsimd.partition_broadcast(m1_b[:], qm1_sb[:])

        for _rep in range(loop_n):
            # q0_sel^T = q0 + is1*(q1 - q0); cast bf16 (g GRU needs it early)
            qsel_bf = []
            for j in range(NDC):
                u = work.tile([128, BL], F32, tag='wu', name='wu')
                nc.vector.tensor_sub(u[:], q1_32[j][:], q0_32[j][:])
                v = work.tile([128, BL], F32, tag='wv', name='wv')
                nc.vector.tensor_mul(v[:], is1_b[:], u[:])
                w32 = work.tile([128, BL], F32, tag='ww', name='ww')
                nc.vector.tensor_add(w32[:], q0_32[j][:], v[:])
                wbf = sb.tile([128, BL], BF, tag=f'qselbf{j}', name=f'qselbf{j}')
                nc.vector.tensor_copy(wbf[:], w32[:])
                qsel_bf.append(wbf)

            # ============ attention + interleaved g GRU ============
            e_sb = [sb.tile([128, T], F32, tag=f'esb{b}', name=f'esb{b}')
                    for b in range(NBLK)]
            sc_sb = [sb.tile([128, T], F32, tag=f'scsb{b}', name=f'scsb{b}')
                     for b in range(NBLK)]
            ctx2 = [sb.tile([128, 2 * BL], BF, tag=f'ctx2{j}', name=f'ctx2{j}')
                    for j in range(NDC)]
            g_zp, g_ng, g_r = [None] * 4, [None] * 4, [None] * 4

            gx = uT + qsel_bf          # g GRU x rhs chunks (K=1024)
            bg = btiles['bg']

            def g_gru_chunk(ci, gps, nps):
                """Emit g-GRU gate-chunk ci inside the attention loop.
                Sigmoids via tanh: sigma(x) = 0.5 + 0.5*tanh(x/2)."""
                def mms(c, ps, klist, xlist, first, last):
                    n = len(klist)
                    for i, (w, x) in enumerate(zip(klist, xlist)):
                        xa = x if isinstance(x, bass.AP) else x[:]
                        nc.tensor.matmul(
                            ps[:], w[:, c * 128:(c + 1) * 128], xa,
                            start=(first and i == 0), stop=(last and i == n - 1))
                if ci < 8:            # r (0..3) / z (4..7) gates
                    c = ci
                    ps = gps.tile([128, BL], F32, tag='ga_ps', name='ga_ps')
                    mms(c, ps, wtiles['gwih'], gx, True, False)
                    mms(c, ps, wtiles['gwhh'], h_bf, False, True)
                    th = work.tile([128, BL], F32, tag='ga_th', name='ga_th')
                    if ci < 4:
                        nc.scalar.activation(th[:], ps[:], ACTF.Tanh,
                                             bias=bg[:, 48 + c:48 + c + 1],
                                             scale=0.5)
                        r = sb.tile([128, BL], F32, tag=f'ga_r{ci}',
                                    name=f'ga_r{ci}')
                        nc.vector.tensor_scalar(out=r[:], in0=th[:],
                                                scalar1=0.5, scalar2=0.5,
                                                op0=ALU.mult, op1=ALU.add)
                        g_r[ci] = r
                    else:
                        nc.scalar.activation(th[:], ps[:], ACTF.Tanh,
                                             bias=bg[:, 60 + c:60 + c + 1],
                                             scale=-0.5)
                        zp = sb.tile([128, BL], F32, tag=f'ga_zp{ci - 4}',
                                     name=f'ga_zp{ci - 4}')
                        nc.vector.tensor_scalar(out=zp[:], in0=th[:],
                                                scalar1=0.5, scalar2=0.5,
                                                op0=ALU.mult, op1=ALU.add)
                        g_zp[ci - 4] = zp
                else:                 # n gate
                    c = ci
                    j = ci - 8
                    ps_i = nps.tile([128, BL], F32, tag='ga_psn_i',
                                    name='ga_psn_i')
                    mms(c, ps_i, wtiles['gwih'], gx, True, True)
                    ps_h = nps.tile([128, BL], F32, tag='ga_psn_h',
                                    name='ga_psn_h')
                    mms(c, ps_h, wtiles['gwhh'], h_bf, True, True)
                    tmp = work.tile([128, BL], F32, tag='ga_tmp', name='ga_tmp')
                    nc.vector.scalar_tensor_tensor(
                        tmp[:], ps_h[:], bg[:, 36 + c:36 + c + 1], g_r[j][:],
                        ALU.add, ALU.mult)
                    tmp2 = work.tile([128, BL], F32, tag='ga_tmp2',
                                     name='ga_tmp2')
                    nc.vector.tensor_add(tmp2[:], tmp[:], ps_i[:])
                    ng = sb.tile([128, BL], F32, tag=f'ga_n{j}', name=f'ga_n{j}')
                    nc.scalar.activation(ng[:], tmp2[:], ACTF.Tanh,
                                         bias=bg[:, 24 + c:24 + c + 1])
                    g_ng[j] = ng

            with tc.tile_pool(name='cpsum', bufs=1, space='PSUM') as cpsum, \
                 tc.tile_pool(name='gapsum', bufs=2, space='PSUM') as gapsum, \
                 tc.tile_pool(name='ganpsum', bufs=1, space='PSUM') as ganpsum, \
                 tc.tile_pool(name='gp', bufs=4) as gp, \
                 tc.tile_pool(name='yp', bufs=2) as yp, \
                 tc.tile_pool(name='dgp', bufs=3) as dgp:
                psum_c = [cpsum.tile([128, D], F32, tag=f'c{b}', name=f'c{b}')
                          for b in range(NBLK)]
                gbig = None
                gsl = [[None] * 4 for _ in range(NBLK)]
                gchunk_sched = {16 + ci * 9: ci for ci in range(NGC)}
                for t in range(T):
                    ti = t % 4
                    if t % 2 == 0:
                        gbig = gp.tile([128, 4 * D], BF, tag='g', name='g')
                        nc.sync.dma_start(
                            gbig[:].rearrange('p (u v d) -> p u v d', u=2, v=2),
                            g[t:t + 2, :, :].rearrange('u (v p) d -> p u v d',
                                                       v=2))
                    for b in range(NBLK):
                        gt = gbig[:, ((t % 2) * 2 + b) * D:
                                  ((t % 2) * 2 + b + 1) * D]
                        gsl[b][ti] = gt
                        idx = t * NBLK + b
                        y = yp.tile([128, D], BF, tag=f'y{b}', name=f'y{b}')
                        if idx % AMR_MOD < AMR_K:
                            nc.vector.affine_mul_reduce(
                                out=y[:], accum_out=sc_sb[b][:, t:t + 1],
                                in0=gt, in1=wb_sb[:], scale=1.0, bias=0.0)
                        else:
                            nc.vector.tensor_mul(y[:], gt, wb_sb[:])
                            nc.scalar.activation(
                                y[:], y[:], ACTF.Copy,
                                accum_out=sc_sb[b][:, t:t + 1])
                    if ti == 3:
                        for b in range(NBLK):
                            nc.scalar.activation(
                                e_sb[b][:, t - 3:t + 1],
                                sc_sb[b][:, t - 3:t + 1], ACTF.Exp)
                            for k in range(4):
                                tt = t - 3 + k
                                dg = dgp.tile([128, 128], BF, tag=f'd{b}',
                                              name=f'd{b}')
                                nc.vector.tensor_scalar_mul(
                                    dg[:], I_sb[:], e_sb[b][:, tt:tt + 1])
                                nc.tensor.matmul(
                                    psum_c[b][:], dg[:], gsl[b][k],
                                    start=(tt == 0), stop=(tt == T - 1))
                    if t in gchunk_sched:
                        g_gru_chunk(gchunk_sched[t], gapsum, ganpsum)

                # softmax normalize; att out; context -> transposed bf16
                with tc.tile_pool(name='tp', bufs=2, space='PSUM') as tpsum:
                    for b in range(NBLK):
                        s = sb.tile([128, 1], F32, tag=f's{b}', name=f's{b}')
                        nc.vector.reduce_sum(s[:], e_sb[b][:], axis=AX)
                        rcp = sb.tile([128, 1], F32, tag=f'rcp{b}',
                                      name=f'rcp{b}')
                        nc.vector.reciprocal(rcp[:], s[:])
                        att_sb = sb.tile([128, T], F32, tag=f'att{b}',
                                         name=f'att{b}')
                        nc.vector.tensor_scalar_mul(att_sb[:], e_sb[b][:],
                                                    rcp[:])
                        nc.sync.dma_start(att_o[b * 128:(b + 1) * 128, :],
                                          att_sb[:])
                        ctx_b = sb.tile([128, D], BF, tag=f'ctxb{b}',
                                        name=f'ctxb{b}')
                        nc.vector.tensor_scalar_mul(ctx_b[:], psum_c[b][:],
                                                    rcp[:])
                        for j in range(NDC):
                            pt = tpsum.tile([128, 128], BF, tag='tr', name='tr')
                            nc.tensor.transpose(
                                pt[:], ctx_b[:, j * 128:(j + 1) * 128], I_sb[:])
                            nc.vector.tensor_copy(
                                ctx2[j][:, b * 128:(b + 1) * 128], pt[:])
                            nc.vector.tensor_copy(
                                ctx2[j][:, BL + b * 128:BL + (b + 1) * 128],
                                pt[:])

            if _rep == 0:
                for nm, kk in [('pwih', 8), ('pwhh', 4)]:
                    tl = []
                    for k in range(kk):
                        w = const.tile([128, G3], BF, tag=f'{nm}{k}',
                                       name=f'{nm}{k}')
                        nc.sync.dma_start(w[:],
                                          wdecl[nm][k * 128:(k + 1) * 128, :])
                        tl.append(w)
                    wtiles[nm] = tl

            # g GRU combine: res_g = h + z'*(n - h)
            for j in range(NDC):
                u = work.tile([128, BL], F32, tag='wu', name='wu')
                nc.vector.tensor_sub(u[:], g_ng[j][:], h_32[j][:])
                v = work.tile([128, BL], F32, tag='wv', name='wv')
                nc.vector.tensor_mul(v[:], g_zp[j][:], u[:])
                r = work.tile([128, BL], F32, tag='ww', name='ww')
                nc.vector.tensor_add(r[:], h_32[j][:], v[:])
                nc.sync.dma_start(rgT_o[j * 128:(j + 1) * 128, :], r[:])

            # ==================== p / e GRU phase ====================
            with tc.tile_pool(name='gpsum', bufs=3, space='PSUM') as gpsum, \
                 tc.tile_pool(name='npsum', bufs=2, space='PSUM') as npsum, \
                 tc.tile_pool(name='ewp', bufs=1) as ewp:
                for nm in ('ewih', 'ewhh'):
                    tl = []
                    for k in range(4):
                        w = ewp.tile([128, G3], BF, tag=f'{nm}{k}',
                                     name=f'{nm}{k}')
                        nc.sync.dma_start(w[:],
                                          wdecl[nm][k * 128:(k + 1) * 128, :])
                        tl.append(w)
                    wtiles[nm] = tl
                e_32 = []
                for j in range(NDC):
                    x = ewp.tile([128, BL], F32, tag=f'e32{j}', name=f'e32{j}')
                    nc.sync.dma_start(x[:], eT32[j * 128:(j + 1) * 128, :])
                    e_32.append(x)
                res_p = [[None] * NDC for _ in range(NP)]
                zp2, ng2 = _emit_gru(nc, sb, gpsum, npsum, wtiles['pwih'],
                                     wtiles['pwhh'], ut2 + ctx2, qp_bf,
                                     btiles['bp'], 2 * BL)
                for p in range(NP):
                    h32 = q0_32 if p == 0 else q1_32
                    mb = m0_b if p == 0 else m1_b
                    psl = slice(p * BL, (p + 1) * BL)
                    for j in range(NDC):
                        u = work.tile([128, BL], F32, tag='wu', name='wu')
                        nc.vector.tensor_sub(u[:], ng2[j][:, psl], h32[j][:])
                        v = work.tile([128, BL], F32, tag='wv', name='wv')
                        nc.vector.tensor_mul(v[:], zp2[j][:, psl], u[:])
                        w2 = work.tile([128, BL], F32, tag='ww', name='ww')
                        nc.vector.tensor_mul(w2[:], mb[:], v[:])
                        r = sb.tile([128, BL], F32, tag=f'resp{p}{j}',
                                    name=f'resp{p}{j}')
                        nc.vector.tensor_add(r[:], h32[j][:], w2[:])
                        res_p[p][j] = r
                        nc.sync.dma_start(rqT_o[p, j * 128:(j + 1) * 128, :],
                                          r[:])

                # sel^T = res0 + is1*(res1 - res0); cast bf16
                sel_bf = []
                for j in range(NDC):
                    u = work.tile([128, BL], F32, tag='wu', name='wu')
                    nc.vector.tensor_sub(u[:], res_p[1][j][:], res_p[0][j][:])
                    v = work.tile([128, BL], F32, tag='wv', name='wv')
                    nc.vector.tensor_mul(v[:], is1_b[:], u[:])
                    w32 = work.tile([128, BL], F32, tag='ww', name='ww')
                    nc.vector.tensor_add(w32[:], res_p[0][j][:], v[:])
                    wbf = sb.tile([128, BL], BF, tag=f'selbf{j}',
                                  name=f'selbf{j}')
                    nc.vector.tensor_copy(wbf[:], w32[:])
                    sel_bf.append(wbf)

                # ---- e GRU: x=sel, h=e; res = e + z'(n-e)
                zp, ng = _emit_gru(nc, sb, gpsum, npsum, wtiles['ewih'],
                                   wtiles['ewhh'], sel_bf, e_bf,
                                   btiles['be'], BL)
                for j in range(NDC):
                    u = work.tile([128, BL], F32, tag='wu', name='wu')
                    nc.vector.tensor_sub(u[:], ng[j][:], e_32[j][:])
                    v = work.tile([128, BL], F32, tag='wv', name='wv')
                    nc.vector.tensor_mul(v[:], zp[j][:], u[:])
                    r = work.tile([128, BL], F32, tag='ww', name='ww')
                    nc.vector.tensor_add(r[:], e_32[j][:], v[:])
                    nc.sync.dma_start(reT_o[j * 128:(j + 1) * 128, :], r[:])

    nc.compile()
    return nc


def _host_prep(utter, q_mask, g_hist, q, e, w_att,
               g_w_ih, g_w_hh, g_b_ih, g_b_hh,
               p_w_ih, p_w_hh, p_b_ih, p_b_hh,
               e_w_ih, e_w_hh, e_b_ih, e_b_hh):
    """Build per-core input maps (layout marshalling only, no model math)."""
    g_bf = np.ascontiguousarray(g_hist).astype(nbf)
    wbc = np.broadcast_to(np.asarray(w_att).astype(nbf), (128, D)).copy()
    ident = np.eye(128, dtype=nbf)

    def packT(x):
        xt = np.ascontiguousarray(np.asarray(x).T)
        return xt.astype(np.float32), xt.astype(nbf)

    q0_32, q0_bf = packT(q[:, 0, :])
    q1_32, q1_bf = packT(q[:, 1, :])
    e_32, e_bf = packT(e)
    h_32, h_bf = packT(np.asarray(g_hist[-1]))
    uT_bf = np.ascontiguousarray(np.asarray(utter).T).astype(nbf)
    qmT = np.ascontiguousarray(np.asarray(q_mask).T).astype(np.float32)

    def wT(w):
        return np.ascontiguousarray(np.asarray(w).T).astype(nbf)

    def bias_pack(b_ih, b_hh):
        s = (np.asarray(b_ih) + np.asarray(b_hh)).reshape(NGC, 128).T
        bi = np.asarray(b_ih).reshape(NGC, 128).T
        bh = np.asarray(b_hh).reshape(NGC, 128).T
        out = np.zeros((128, 72), np.float32)
        out[:, 0:12] = s
        out[:, 12:24] = -s
        out[:, 24:36] = bi
        out[:, 36:48] = bh
        out[:, 48:60] = 0.5 * s
        out[:, 60:72] = -0.5 * s
        return out

    shared = {
        'wb': wbc, 'ident': ident,
        'gwih': wT(g_w_ih), 'gwhh': wT(g_w_hh),
        'pwih': wT(p_w_ih), 'pwhh': wT(p_w_hh),
        'ewih': wT(e_w_ih), 'ewhh': wT(e_w_hh),
        'bg': bias_pack(g_b_ih, g_b_hh),
        'bp': bias_pack(p_b_ih, p_b_hh),
        'be': bias_pack(e_b_ih, e_b_hh),
    }
    in_maps = []
    for c in range(N_CORES):
        sl = slice(c * BL, (c + 1) * BL)
        m = dict(shared)
        m.update({
            'g': np.ascontiguousarray(g_bf[:, sl, :]),
            'utT2': np.concatenate([uT_bf[:, sl], uT_bf[:, sl]], axis=1),
            'qpTbf': np.concatenate([q0_bf[:, sl], q1_bf[:, sl]], axis=1),
            'q0T32': np.ascontiguousarray(q0_32[:, sl]),
            'q1T32': np.ascontiguousarray(q1_32[:, sl]),
            'eT32': np.ascontiguousarray(e_32[:, sl]),
            'eTbf': np.ascontiguousarray(e_bf[:, sl]),
            'hT32': np.ascontiguousarray(h_32[:, sl]),
            'hTbf': np.ascontiguousarray(h_bf[:, sl]),
            'qm0': np.ascontiguousarray(qmT[0:1, sl]),
            'qm1': np.ascontiguousarray(qmT[1:2, sl]),
        })
        in_maps.append(m)
    return in_maps


def kernel(**inputs):
    inputs = {k: np.asarray(v) for k, v in inputs.items()}
    in_maps = _host_prep(**inputs)
    if 'nc' not in _compiled:
        _compiled['nc'] = _build()
    nc = _compiled['nc']
    res = run_bass_kernel_spmd(nc, in_maps, list(range(N_CORES))).results

    res_g = np.concatenate([r['rgT_o'].T for r in res], axis=0)
    res_q = np.concatenate([np.transpose(r['rqT_o'], (2, 0, 1)) for r in res],
                           axis=0)
    res_e = np.concatenate([r['reT_o'].T for r in res], axis=0)
    att = np.concatenate([r['att_o'][:, None, :] for r in res], axis=0)
    return (np.ascontiguousarray(res_g, dtype=np.float32),
            np.ascontiguousarray(res_q, dtype=np.float32),
            np.ascontiguousarray(res_e, dtype=np.float32),
            np.ascontiguousarray(att, dtype=np.float32))


# revision 12
# speedup vs baseline: 1.2677x; 1.2260x over previous
"""DialogueRNNCell fused Bass/Tile kernel for 8 Trainium2 NeuronCores.

Pure data parallel over the batch dim (2048 -> 256 rows/core). Per core, a
single fused pass over its g_hist shard computes the attention:
  scale_t = G_t . w       -- DVE affine_mul_reduce, or DVE mul + ACT accum
                             (split between engines to balance load)
  e_t = exp(scale_t)      -- ACT (exp table set)
  context += diag(e_t)^T @ G_t  -- tensor engine PSUM accumulation
The g GRU is interleaved into the attention loop (its sigmoids are computed
as 0.5+0.5*tanh(x/2) so the ACT engine stays on the exp/tanh table set).
The p GRU processes both parties in one set of 512-wide matmuls; the e GRU
follows. GRU weights are streamed from HBM one gate-chunk at a time (each
weight byte is used exactly once), keeping SBUF free for deep pipelining.
GRU combines are algebraically fused:
  h' = h + z'*(n - h),  z' = 1 - z;  party blend: res = q + m*z'*(n - q).
"""

import numpy as np
import ml_dtypes

import concourse.bass as bass
import concourse.mybir as mybir
import concourse.tile as tile
from concourse import bacc
from concourse.bass_utils import run_bass_kernel_spmd

B, T, NP, D = 2048, 128, 2, 512
N_CORES = 8
BL = B // N_CORES          # rows per core (256)
NBLK = BL // 128           # 128-row blocks per core (2)
NDC = D // 128             # feature chunks of 128 (4)
G3 = 3 * D                 # stacked gate rows (r, z, n) = 1536
NGC = G3 // 128            # gate chunks (12)

# scale-path split: tile idx % AMR_MOD < AMR_K -> affine_mul_reduce on DVE,
# else DVE mul + ACT copy-accum. Tuned so DVE and ACT finish together.
AMR_K, AMR_MOD = 6, 13

BF = mybir.dt.bfloat16
F32 = mybir.dt.float32
nbf = ml_dtypes.bfloat16

AX = mybir.AxisListType.X
ALU = mybir.AluOpType
ACTF = mybir.ActivationFunctionType

_compiled = {}


def _emit_gru(nc, sb, work, rz_psum, n_psum, wkp, wpk_dram, nk, x_chunks,
              h_chunks, bpack, bl, tanh_form=False):
    """One GRU cell in transposed layout with streamed weights.

    wpk_dram: [NGC, 128, nk, 128] packed weight chunks (ih k-chunks then hh).
    x_chunks + h_chunks: rhs tiles/APs, len == nk, zipped against k slices.
    bpack cols: 0:12 s, 12:24 -s, 24:36 b_ih, 36:48 b_hh, 48:60 s/2,
    60:72 -s/2 (s = b_ih + b_hh). tanh_form: sigmoid via exp-set tanh.
    Returns (zp_chunks, n_chunks).
    """
    rhs = list(x_chunks) + list(h_chunks)

    def load_w(c):
        wt = wkp.tile([128, nk * 128], BF, tag='wk', name='wk')
        nc.sync.dma_start(
            wt[:].rearrange('p (k g) -> p k g', k=nk), wpk_dram[c])
        return wt

    def mms(c, wt, ps):
        for k in range(nk):
            xa = rhs[k] if isinstance(rhs[k], bass.AP) else rhs[k][:]
            nc.tensor.matmul(ps[:], wt[:, k * 128:(k + 1) * 128], xa,
                             start=(k == 0), stop=(k == nk - 1))

    def sigmoid(out_tag, ps, c, neg):
        if tanh_form:
            bcol = bpack[:, (60 if neg else 48) + c:(60 if neg else 48) + c + 1]
            th = work.tile([128, bl], F32, tag='th', name='th')
            nc.scalar.activation(th[:], ps[:], ACTF.Tanh, bias=bcol,
                                 scale=(-0.5 if neg else 0.5))
            o = sb.tile([128, bl], F32, tag=out_tag, name=out_tag)
            nc.vector.tensor_scalar(out=o[:], in0=th[:], scalar1=0.5,
                                    scalar2=0.5, op0=ALU.mult, op1=ALU.add)
        else:
            bcol = bpack[:, (12 if neg else 0) + c:(12 if neg else 0) + c + 1]
            o = sb.tile([128, bl], F32, tag=out_tag, name=out_tag)
            nc.scalar.activation(o[:], ps[:], ACTF.Sigmoid, bias=bcol,
                                 scale=(-1.0 if neg else 1.0))
        return o

    emitted = []

    def chunk(ci):
        wt = load_w(ci)
        if ci < 4:                         # r gate
            ps = rz_psum.tile([128, bl], F32, tag='gru_ps', name='gru_ps')
            mms(ci, wt, ps)
            emitted.append(('r', ci, sigmoid(f'gru_r{ci}', ps, ci, False)))
        elif ci < 8:                       # z gate -> z'
            ps = rz_psum.tile([128, bl], F32, tag='gru_ps', name='gru_ps')
            mms(ci, wt, ps)
            emitted.append(('zp', ci - 4, sigmoid(f'gru_zp{ci - 4}', ps, ci,
                                                  True)))
        else:                              # n gate
            c = ci
            j = ci - 8
            ps_i = n_psum.tile([128, bl], F32, tag='gru_psn_i', name='gru_psn_i')
            for k in range(nk - len(h_chunks)):
                xa = rhs[k] if isinstance(rhs[k], bass.AP) else rhs[k][:]
                nc.tensor.matmul(ps_i[:], wt[:, k * 128:(k + 1) * 128], xa,
                                 start=(k == 0),
                                 stop=(k == nk - len(h_chunks) - 1))
            ps_h = n_psum.tile([128, bl], F32, tag='gru_psn_h', name='gru_psn_h')
            kh0 = nk - len(h_chunks)
            for i, k in enumerate(range(kh0, nk)):
                xa = rhs[k] if isinstance(rhs[k], bass.AP) else rhs[k][:]
                nc.tensor.matmul(ps_h[:], wt[:, k * 128:(k + 1) * 128], xa,
                                 start=(i == 0), stop=(k == nk - 1))
            rtile = next(x for kind, jj, x in emitted if kind == 'r' and jj == j)
            tmp = work.tile([128, bl], F32, tag='gru_tmp', name='gru_tmp')
            nc.vector.scalar_tensor_tensor(    # (gh_n + b_hh) * r
                tmp[:], ps_h[:], bpack[:, 36 + c:36 + c + 1], rtile[:],
                ALU.add, ALU.mult)
            tmp2 = work.tile([128, bl], F32, tag='gru_tmp2', name='gru_tmp2')
            nc.vector.tensor_add(tmp2[:], tmp[:], ps_i[:])
            ng = sb.tile([128, bl], F32, tag=f'gru_n{j}', name=f'gru_n{j}')
            nc.scalar.activation(ng[:], tmp2[:], ACTF.Tanh,
                                 bias=bpack[:, 24 + c:24 + c + 1])
            emitted.append(('n', j, ng))

    for ci in range(NGC):
        chunk(ci)
    zp = [x for kind, j, x in emitted if kind == 'zp']
    ng = [x for kind, j, x in emitted if kind == 'n']
    return zp, ng


def _build(loop_n=1):
    nc = bacc.Bacc('TRN2', target_bir_lowering=False, debug=False,
                   num_devices=N_CORES)
    # ---- inputs (per-core shapes) ----
    g = nc.dram_tensor('g', [T, BL, D], BF, kind='ExternalInput')
    wb = nc.dram_tensor('wb', [128, D], BF, kind='ExternalInput')
    ident = nc.dram_tensor('ident', [128, 128], BF, kind='ExternalInput')
    utT2 = nc.dram_tensor('utT2', [D, 2 * BL], BF, kind='ExternalInput')
    qpTbf = nc.dram_tensor('qpTbf', [D, 2 * BL], BF, kind='ExternalInput')
    q0T32 = nc.dram_tensor('q0T32', [D, BL], F32, kind='ExternalInput')
    q1T32 = nc.dram_tensor('q1T32', [D, BL], F32, kind='ExternalInput')
    eT32 = nc.dram_tensor('eT32', [D, BL], F32, kind='ExternalInput')
    eTbf = nc.dram_tensor('eTbf', [D, BL], BF, kind='ExternalInput')
    hT32 = nc.dram_tensor('hT32', [D, BL], F32, kind='ExternalInput')
    hTbf = nc.dram_tensor('hTbf', [D, BL], BF, kind='ExternalInput')
    qm0 = nc.dram_tensor('qm0', [1, BL], F32, kind='ExternalInput')
    qm1 = nc.dram_tensor('qm1', [1, BL], F32, kind='ExternalInput')
    # packed streamed weights: [chunk, K(128), k, gate(128)]
    gwpk = nc.dram_tensor('gwpk', [NGC, 128, 12, 128], BF, kind='ExternalInput')
    pwpk = nc.dram_tensor('pwpk', [NGC, 128, 12, 128], BF, kind='ExternalInput')
    ewpk = nc.dram_tensor('ewpk', [NGC, 128, 8, 128], BF, kind='ExternalInput')
    bdecl = {nm: nc.dram_tensor(nm, [128, 72], F32, kind='ExternalInput')
             for nm in ('bg', 'bp', 'be')}
    # ---- outputs ----
    att_o = nc.dram_tensor('att_o', [BL, T], F32, kind='ExternalOutput')
    rgT_o = nc.dram_tensor('rgT_o', [D, BL], F32, kind='ExternalOutput')
    rqT_o = nc.dram_tensor('rqT_o', [NP, D, BL], F32, kind='ExternalOutput')
    reT_o = nc.dram_tensor('reT_o', [D, BL], F32, kind='ExternalOutput')

    from contextlib import ExitStack
    with tile.TileContext(nc) as tc, ExitStack() as top:
        const = top.enter_context(tc.tile_pool(name='const', bufs=1))
        sb = top.enter_context(tc.tile_pool(name='sb', bufs=1))
        work = top.enter_context(tc.tile_pool(name='work', bufs=3))
        wkp = top.enter_context(tc.tile_pool(name='wkp', bufs=3))

        # ---- persistent constants ----
        wb_sb = const.tile([128, D], BF, tag='wb', name='wb')
        nc.sync.dma_start(wb_sb[:], wb[:])
        I_sb = const.tile([128, 128], BF, tag='ident', name='ident')
        nc.sync.dma_start(I_sb[:], ident[:])
        btiles = {}
        for nm in ('bg', 'bp', 'be'):
            bt = const.tile([128, 72], F32, tag=nm)
            nc.sync.dma_start(bt[:], bdecl[nm][:])
            btiles[nm] = bt

        def load_T(dram, dt, tagp, width=BL):
            tl = []
            for j in range(NDC):
                x = const.tile([128, width], dt, tag=f'{tagp}{j}',
                               name=f'{tagp}{j}')
                nc.sync.dma_start(x[:], dram[j * 128:(j + 1) * 128, :])
                tl.append(x)
            return tl

        ut2 = load_T(utT2, BF, 'ut2', 2 * BL)
        qp_bf = load_T(qpTbf, BF, 'qpbf', 2 * BL)
        uT = [x[:, 0:BL] for x in ut2]
        q0_32 = load_T(q0T32, F32, 'q032')
        q1_32 = load_T(q1T32, F32, 'q132')
        e_32 = load_T(eT32, F32, 'e32')
        e_bf = load_T(eTbf, BF, 'ebf')
        h_32 = load_T(hT32, F32, 'h32')
        h_bf = load_T(hTbf, BF, 'hbf')

        qm0_sb = const.tile([1, BL], F32, tag='qm0', name='qm0')
        nc.sync.dma_start(qm0_sb[:], qm0[:])
        qm1_sb = const.tile([1, BL], F32, tag='qm1', name='qm1')
        nc.sync.dma_start(qm1_sb[:], qm1[:])
        is1_row = const.tile([1, BL], F32, tag='is1row', name='is1row')
        nc.vector.tensor_tensor(out=is1_row[:], in0=qm1_sb[:], in1=qm0_sb[:],
                                op=ALU.is_gt)
        is1_b = const.tile([128, BL], F32, tag='is1b', name='is1b')
        nc.gpsimd.partition_broadcast(is1_b[:], is1_row[:])
        m0_b = const.tile([128, BL], F32, tag='m0b', name='m0b')
        nc.gpsimd.partition_broadcast(m0_b[:], qm0_sb[:])
        m1_b = const.tile([128, BL], F32, tag='m1b', name='m1b')
        nc.gpsimd.partition_broadcast(m1_b[:], qm1_sb[:])

        for _rep in range(loop_n):
            # q0_sel^T = q0 + is1*(q1 - q0); cast bf16 (g GRU needs it early)
            qsel_bf = []
            for j in range(NDC):
                u = work.tile([128, BL], F32, tag='wu', name='wu')
                nc.vector.tensor_sub(u[:], q1_32[j][:], q0_32[j][:])
                v = work.tile([128, BL], F32, tag='wv', name='wv')
                nc.vector.tensor_mul(v[:], is1_b[:], u[:])
                w32 = work.tile([128, BL], F32, tag='ww', name='ww')
                nc.vector.tensor_add(w32[:], q0_32[j][:], v[:])
                wbf = sb.tile([128, BL], BF, tag=f'qselbf{j}', name=f'qselbf{j}')
                nc.vector.tensor_copy(wbf[:], w32[:])
                qsel_bf.append(wbf)

            # ============ attention + interleaved g GRU ============
            e_sb = [sb.tile([128, T], F32, tag=f'esb{b}', name=f'esb{b}')
                    for b in range(NBLK)]
            sc_sb = [sb.tile([128, T], F32, tag=f'scsb{b}', name=f'scsb{b}')
                     for b in range(NBLK)]
            ctx2 = [sb.tile([128, 2 * BL], BF, tag=f'ctx2{j}', name=f'ctx2{j}')
                    for j in range(NDC)]
            g_zp, g_ng, g_r = [None] * 4, [None] * 4, [None] * 4

            gx = uT + qsel_bf          # g GRU x rhs chunks (K=1024)
            grhs = gx + h_bf
            bg = btiles['bg']

            def g_load_w(ci):
                wt = wkp.tile([128, 12 * 128], BF, tag='wk', name='wk')
                nc.sync.dma_start(
                    wt[:].rearrange('p (k g) -> p k g', k=12), gwpk[ci])
                return wt

            def g_gru_chunk(ci, wt, gps, nps):
                """g-GRU gate-chunk ci inside the attention loop; sigmoids
                via tanh so ACT stays on the exp table set."""
                def mms(ps, ks, ke):
                    for i, k in enumerate(range(ks, ke)):
                        xa = (grhs[k] if isinstance(grhs[k], bass.AP)
                              else grhs[k][:])
                        nc.tensor.matmul(ps[:], wt[:, k * 128:(k + 1) * 128],
                                         xa, start=(i == 0),
                                         stop=(k == ke - 1))
                if ci < 8:            # r (0..3) / z (4..7) gates
                    c = ci
                    ps = gps.tile([128, BL], F32, tag='ga_ps', name='ga_ps')
                    mms(ps, 0, 12)
                    th = work.tile([128, BL], F32, tag='ga_th', name='ga_th')
                    if ci < 4:
                        nc.scalar.activation(th[:], ps[:], ACTF.Tanh,
                                             bias=bg[:, 48 + c:48 + c + 1],
                                             scale=0.5)
                        r = sb.tile([128, BL], F32, tag=f'ga_r{ci}',
                                    name=f'ga_r{ci}')
                        nc.vector.tensor_scalar(out=r[:], in0=th[:],
                                                scalar1=0.5, scalar2=0.5,
                                                op0=ALU.mult, op1=ALU.add)
                        g_r[ci] = r
                    else:
                        nc.scalar.activation(th[:], ps[:], ACTF.Tanh,
                                             bias=bg[:, 60 + c:60 + c + 1],
                                             scale=-0.5)
                        zp = sb.tile([128, BL], F32, tag=f'ga_zp{ci - 4}',
                                     name=f'ga_zp{ci - 4}')
                        nc.vector.tensor_scalar(out=zp[:], in0=th[:],
                                                scalar1=0.5, scalar2=0.5,
                                                op0=ALU.mult, op1=ALU.add)
                        g_zp[ci - 4] = zp
                else:                 # n gate
                    c = ci
                    j = ci - 8
                    ps_i = nps.tile([128, BL], F32, tag='ga_psn_i',
                                    name='ga_psn_i')
                    mms(ps_i, 0, 8)
                    ps_h = nps.tile([128, BL], F32, tag='ga_psn_h',
                                    name='ga_psn_h')
                    mms(ps_h, 8, 12)
                    tmp = work.tile([128, BL], F32, tag='ga_tmp', name='ga_tmp')
                    nc.vector.scalar_tensor_tensor(
                        tmp[:], ps_h[:], bg[:, 36 + c:36 + c + 1], g_r[j][:],
                        ALU.add, ALU.mult)
                    tmp2 = work.tile([128, BL], F32, tag='ga_tmp2',
                                     name='ga_tmp2')
                    nc.vector.tensor_add(tmp2[:], tmp[:], ps_i[:])
                    ng = sb.tile([128, BL], F32, tag=f'ga_n{j}', name=f'ga_n{j}')
                    nc.scalar.activation(ng[:], tmp2[:], ACTF.Tanh,
                                         bias=bg[:, 24 + c:24 + c + 1])
                    g_ng[j] = ng

            with tc.tile_pool(name='cpsum', bufs=1, space='PSUM') as cpsum, \
                 tc.tile_pool(name='gapsum', bufs=2, space='PSUM') as gapsum, \
                 tc.tile_pool(name='ganpsum', bufs=1, space='PSUM') as ganpsum, \
                 tc.tile_pool(name='gp', bufs=10) as gp, \
                 tc.tile_pool(name='yp', bufs=4) as yp, \
                 tc.tile_pool(name='dgp', bufs=4) as dgp:
                psum_c = [cpsum.tile([128, D], F32, tag=f'c{b}', name=f'c{b}')
                          for b in range(NBLK)]
                gbig = None
                g_wt = None
                gsl = [[None] * 4 for _ in range(NBLK)]
                gchunk_sched = {16 + ci * 9: ci for ci in range(NGC)}
                gload_sched = {8 + ci * 9: ci for ci in range(NGC)}
                for t in range(T):
                    ti = t % 4
                    if t % 2 == 0:
                        gbig = gp.tile([128, 4 * D], BF, tag='g', name='g')
                        nc.sync.dma_start(
                            gbig[:].rearrange('p (u v d) -> p u v d', u=2, v=2),
                            g[t:t + 2, :, :].rearrange('u (v p) d -> p u v d',
                                                       v=2))
                    for b in range(NBLK):
                        gt = gbig[:, ((t % 2) * 2 + b) * D:
                                  ((t % 2) * 2 + b + 1) * D]
                        gsl[b][ti] = gt
                        idx = t * NBLK + b
                        y = yp.tile([128, D], BF, tag=f'y{b}', name=f'y{b}')
                        if idx % AMR_MOD < AMR_K:
                            nc.vector.affine_mul_reduce(
                                out=y[:], accum_out=sc_sb[b][:, t:t + 1],
                                in0=gt, in1=wb_sb[:], scale=1.0, bias=0.0)
                        else:
                            nc.vector.tensor_mul(y[:], gt, wb_sb[:])
                            nc.scalar.activation(
                                y[:], y[:], ACTF.Copy,
                                accum_out=sc_sb[b][:, t:t + 1])
                    if ti == 3:
                        for b in range(NBLK):
                            nc.scalar.activation(
                                e_sb[b][:, t - 3:t + 1],
                                sc_sb[b][:, t - 3:t + 1], ACTF.Exp)
                            for k in range(4):
                                tt = t - 3 + k
                                dg = dgp.tile([128, 128], BF, tag=f'd{b}',
                                              name=f'd{b}')
                                nc.vector.tensor_scalar_mul(
                                    dg[:], I_sb[:], e_sb[b][:, tt:tt + 1])
                                nc.tensor.matmul(
                                    psum_c[b][:], dg[:], gsl[b][k],
                                    start=(tt == 0), stop=(tt == T - 1))
                    if t in gload_sched:
                        g_wt = g_load_w(gload_sched[t])
                    if t in gchunk_sched:
                        g_gru_chunk(gchunk_sched[t], g_wt, gapsum, ganpsum)

                # softmax normalize; att out; context -> transposed bf16 pairs
                with tc.tile_pool(name='tp', bufs=2, space='PSUM') as tpsum:
                    for b in range(NBLK):
                        s = sb.tile([128, 1], F32, tag=f's{b}', name=f's{b}')
                        nc.vector.reduce_sum(s[:], e_sb[b][:], axis=AX)
                        rcp = sb.tile([128, 1], F32, tag=f'rcp{b}',
                                      name=f'rcp{b}')
                        nc.vector.reciprocal(rcp[:], s[:])
                        att_sb = sb.tile([128, T], F32, tag=f'att{b}',
                                         name=f'att{b}')
                        nc.vector.tensor_scalar_mul(att_sb[:], e_sb[b][:],
                                                    rcp[:])
                        nc.sync.dma_start(att_o[b * 128:(b + 1) * 128, :],
                                          att_sb[:])
                        ctx_b = sb.tile([128, D], BF, tag=f'ctxb{b}',
                                        name=f'ctxb{b}')
                        nc.vector.tensor_scalar_mul(ctx_b[:], psum_c[b][:],
                                                    rcp[:])
                        for j in range(NDC):
                            pt = tpsum.tile([128, 128], BF, tag='tr', name='tr')
                            nc.tensor.transpose(
                                pt[:], ctx_b[:, j * 128:(j + 1) * 128], I_sb[:])
                            nc.vector.tensor_copy(
                                ctx2[j][:, b * 128:(b + 1) * 128], pt[:])
                            nc.vector.tensor_copy(
                                ctx2[j][:, BL + b * 128:BL + (b + 1) * 128],
                                pt[:])

            # g GRU combine: res_g = h + z'*(n - h)
            for j in range(NDC):
                u = work.tile([128, BL], F32, tag='wu', name='wu')
                nc.vector.tensor_sub(u[:], g_ng[j][:], h_32[j][:])
                v = work.tile([128, BL], F32, tag='wv', name='wv')
                nc.vector.tensor_mul(v[:], g_zp[j][:], u[:])
                r = work.tile([128, BL], F32, tag='ww', name='ww')
                nc.vector.tensor_add(r[:], h_32[j][:], v[:])
                nc.sync.dma_start(rgT_o[j * 128:(j + 1) * 128, :], r[:])

            # ==================== p / e GRU phase ====================
            with tc.tile_pool(name='gpsum', bufs=3, space='PSUM') as gpsum, \
                 tc.tile_pool(name='npsum', bufs=2, space='PSUM') as npsum:
                res_p = [[None] * NDC for _ in range(NP)]
                zp2, ng2 = _emit_gru(nc, sb, work, gpsum, npsum, wkp, pwpk,
                                     12, ut2 + ctx2, qp_bf, btiles['bp'],
                                     2 * BL)
                for p in range(NP):
                    h32 = q0_32 if p == 0 else q1_32
                    mb = m0_b if p == 0 else m1_b
                    psl = slice(p * BL, (p + 1) * BL)
                    for j in range(NDC):
                        u = work.tile([128, BL], F32, tag='wu', name='wu')
                        nc.vector.tensor_sub(u[:], ng2[j][:, psl], h32[j][:])
                        v = work.tile([128, BL], F32, tag='wv', name='wv')
                        nc.vector.tensor_mul(v[:], zp2[j][:, psl], u[:])
                        w2 = work.tile([128, BL], F32, tag='ww', name='ww')
                        nc.vector.tensor_mul(w2[:], mb[:], v[:])
                        r = sb.tile([128, BL], F32, tag=f'resp{p}{j}',
                                    name=f'resp{p}{j}')
                        nc.vector.tensor_add(r[:], h32[j][:], w2[:])
                        res_p[p][j] = r
                        nc.sync.dma_start(rqT_o[p, j * 128:(j + 1) * 128, :],
                                          r[:])

                # sel^T = res0 + is1*(res1 - res0); cast bf16
                sel_bf = []
                for j in range(NDC):
                    u = work.tile([128, BL], F32, tag='wu', name='wu')
                    nc.vector.tensor_sub(u[:], res_p[1][j][:], res_p[0][j][:])
                    v = work.tile([128, BL], F32, tag='wv', name='wv')
                    nc.vector.tensor_mul(v[:], is1_b[:], u[:])
                    w32 = work.tile([128, BL], F32, tag='ww', name='ww')
                    nc.vector.tensor_add(w32[:], res_p[0][j][:], v[:])
                    wbf = sb.tile([128, BL], BF, tag=f'selbf{j}',
                                  name=f'selbf{j}')
                    nc.vector.tensor_copy(wbf[:], w32[:])
                    sel_bf.append(wbf)

                # ---- e GRU: x=sel, h=e; res = e + z'(n-e)
                zp, ng = _emit_gru(nc, sb, work, gpsum, npsum, wkp, ewpk,
                                   8, sel_bf, e_bf, btiles['be'], BL)
                for j in range(NDC):
                    u = work.tile([128, BL], F32, tag='wu', name='wu')
                    nc.vector.tensor_sub(u[:], ng[j][:], e_32[j][:])
                    v = work.tile([128, BL], F32, tag='wv', name='wv')
                    nc.vector.tensor_mul(v[:], zp[j][:], u[:])
                    r = work.tile([128, BL], F32, tag='ww', name='ww')
                    nc.vector.tensor_add(r[:], e_32[j][:], v[:])
                    nc.sync.dma_start(reT_o[j * 128:(j + 1) * 128, :], r[:])

    nc.compile()
    return nc


def _host_prep(utter, q_mask, g_hist, q, e, w_att,
               g_w_ih, g_w_hh, g_b_ih, g_b_hh,
               p_w_ih, p_w_hh, p_b_ih, p_b_hh,
               e_w_ih, e_w_hh, e_b_ih, e_b_hh):
    """Build per-core input maps (layout marshalling only, no model math)."""
    g_bf = np.ascontiguousarray(g_hist).astype(nbf)
    wbc = np.broadcast_to(np.asarray(w_att).astype(nbf), (128, D)).copy()
    ident = np.eye(128, dtype=nbf)

    def packT(x):
        xt = np.ascontiguousarray(np.asarray(x).T)
        return xt.astype(np.float32), xt.astype(nbf)

    q0_32, q0_bf = packT(q[:, 0, :])
    q1_32, q1_bf = packT(q[:, 1, :])
    e_32, e_bf = packT(e)
    h_32, h_bf = packT(np.asarray(g_hist[-1]))
    uT_bf = np.ascontiguousarray(np.asarray(utter).T).astype(nbf)
    qmT = np.ascontiguousarray(np.asarray(q_mask).T).astype(np.float32)

    def wpack(w_ih, w_hh):
        """[NGC, 128(K), nk, 128(gate)] with ih k-chunks then hh k-chunks."""
        blocks = []
        for w in (w_ih, w_hh):
            w = np.asarray(w)
            nk = w.shape[1] // 128
            # [c, gate, k, K] -> [c, K, k, gate]
            r = w.reshape(NGC, 128, nk, 128).transpose(0, 3, 2, 1)
            blocks.append(r)
        return np.ascontiguousarray(
            np.concatenate(blocks, axis=2)).astype(nbf)

    def bias_pack(b_ih, b_hh):
        s = (np.asarray(b_ih) + np.asarray(b_hh)).reshape(NGC, 128).T
        bi = np.asarray(b_ih).reshape(NGC, 128).T
        bh = np.asarray(b_hh).reshape(NGC, 128).T
        out = np.zeros((128, 72), np.float32)
        out[:, 0:12] = s
        out[:, 12:24] = -s
        out[:, 24:36] = bi
        out[:, 36:48] = bh
        out[:, 48:60] = 0.5 * s
        out[:, 60:72] = -0.5 * s
        return out

    shared = {
        'wb': wbc, 'ident': ident,
        'gwpk': wpack(g_w_ih, g_w_hh),
        'pwpk': wpack(p_w_ih, p_w_hh),
        'ewpk': wpack(e_w_ih, e_w_hh),
        'bg': bias_pack(g_b_ih, g_b_hh),
        'bp': bias_pack(p_b_ih, p_b_hh),
        'be': bias_pack(e_b_ih, e_b_hh),
    }
    in_maps = []
    for c in range(N_CORES):
        sl = slice(c * BL, (c + 1) * BL)
        m = dict(shared)
        m.update({
            'g': np.ascontiguousarray(g_bf[:, sl, :]),
            'utT2': np.concatenate([uT_bf[:, sl], uT_bf[:, sl]], axis=1),
            'qpTbf': np.concatenate([q0_bf[:, sl], q1_bf[:, sl]], axis=1),
            'q0T32': np.ascontiguousarray(q0_32[:, sl]),
            'q1T32': np.ascontiguousarray(q1_32[:, sl]),
            'eT32': np.ascontiguousarray(e_32[:, sl]),
            'eTbf': np.ascontiguousarray(e_bf[:, sl]),
            'hT32': np.ascontiguousarray(h_32[:, sl]),
            'hTbf': np.ascontiguousarray(h_bf[:, sl]),
            'qm0': np.ascontiguousarray(qmT[0:1, sl]),
            'qm1': np.ascontiguousarray(qmT[1:2, sl]),
        })
        in_maps.append(m)
    return in_maps


def kernel(**inputs):
    inputs = {k: np.asarray(v) for k, v in inputs.items()}
    in_maps = _host_prep(**inputs)
    if 'nc' not in _compiled:
        _compiled['nc'] = _build()
    nc = _compiled['nc']
    res = run_bass_kernel_spmd(nc, in_maps, list(range(N_CORES))).results

    res_g = np.concatenate([r['rgT_o'].T for r in res], axis=0)
    res_q = np.concatenate([np.transpose(r['rqT_o'], (2, 0, 1)) for r in res],
                           axis=0)
    res_e = np.concatenate([r['reT_o'].T for r in res], axis=0)
    att = np.concatenate([r['att_o'][:, None, :] for r in res], axis=0)
    return (np.ascontiguousarray(res_g, dtype=np.float32),
            np.ascontiguousarray(res_q, dtype=np.float32),
            np.ascontiguousarray(res_e, dtype=np.float32),
            np.ascontiguousarray(att, dtype=np.float32))
